# revision 2
# baseline (speedup 1.0000x reference)
"""Trainium2 Bass kernel for 3x3 same-padding Conv2d (B=16, C=256, H=W=112) + bias.

Strategy: data-parallel over batch (2 images per core on 8 NeuronCores).
Per core, implicit GEMM: activations live in SBUF in a zero-padded layout with
row stride 113 (one shared zero column between rows plus guard rows), so each
of the 9 conv taps is a matmul whose moving operand is the same buffer sliced
at a shifted offset. Contraction over 256 input channels = 2 partition tiles;
256 output channels = 2 PSUM tiles. Matmuls run in float32r (TF32-like,
1 cycle/row at N>=256, ~1.5e-4 rel err) fed directly by DMA.

The image is processed in 4 horizontal strips of 28 output rows (30 input
rows with halo). Each strip yields 7 PSUM accumulation groups of 452
positions (4 output rows x 113) per output-channel half; each group
accumulates 18 matmuls (9 taps x 2 input-channel halves). Bias is fused into
the PSUM->SBUF drain via tensor_scalar_add.
"""
import numpy as np

from concourse import bacc, bass, mybir, tile
from concourse.bass_utils import run_bass_kernel_spmd

B, C, H, Wd = 16, 256, 112, 112
NCORES = 8
BPC = B // NCORES        # images per core
S = 113                  # padded row stride (112 data + 1 shared zero col)
G = 4                    # guard zeros at buffer start
RS = 28                  # output rows per strip
NSTRIP = H // RS         # 4
IN_ROWS = RS + 2         # input rows per strip incl. halo
BUFLEN = G + IN_ROWS * S + 2   # 3396
NPOS = 4 * S             # 452 positions per matmul / psum group
NT = RS // 4             # 7 psum groups per strip
f32 = mybir.dt.float32
f32r = mybir.dt.float32r


def build(repeat: int = 1):
    nc = bacc.Bacc("TRN2", debug=False)
    inp_d = nc.dram_tensor("inp", [BPC, C, H, Wd], f32, kind="ExternalInput").ap()
    w_d = nc.dram_tensor("w", [3, 3, C, C], f32, kind="ExternalInput").ap()
    bias_d = nc.dram_tensor("bias", [C, 1], f32, kind="ExternalInput").ap()
    out_d = nc.dram_tensor("out", [BPC, C, H, Wd], f32, kind="ExternalOutput").ap()

    with tile.TileContext(nc) as tc:
        with (
            tc.tile_pool(name="wpool", bufs=1) as wp,
            tc.tile_pool(name="ipool", bufs=1) as ip,
            tc.tile_pool(name="opool", bufs=8) as op,
            tc.tile_pool(name="pspool", bufs=8, space=bass.MemorySpace.PSUM) as pp,
        ):
            # Weight tiles: lhsT[K=in_ch, M=out_ch] per (tap, in-half, out-half).
            wt = {}
            for ky in range(3):
                for kx in range(3):
                    for kh in range(2):
                        for mh in range(2):
                            t = wp.tile([128, 128], f32r, name=f"w_{ky}{kx}{kh}{mh}")
                            nc.sync.dma_start(
                                t[:],
                                w_d[ky, kx, kh * 128:(kh + 1) * 128,
                                    mh * 128:(mh + 1) * 128].bitcast(f32r),
                            )
                            wt[ky, kx, kh, mh] = t
            biases = []
            for mh in range(2):
                bt = wp.tile([128, 1], f32, name=f"bias_{mh}")
                nc.sync.dma_start(bt[:], bias_d[mh * 128:(mh + 1) * 128, :])
                biases.append(bt)

            # Input buffers: fixed per (strip, in-half) so the zero borders
            # (guards, shared zero columns, halo rows of first/last strip)
            # survive reuse across images; DMA only ever writes the interior.
            # Zeroing goes through a broadcast f32->f32r copy: walrus rejects
            # Memset on float32r, and every producer feeding an f32r matmul
            # must itself emit f32r ("rounded") output.
            zt = wp.tile([128, 1], f32, name="zero_src")
            nc.vector.memset(zt[:], 0.0)
            bufs = {}
            for s in range(NSTRIP):
                for kh in range(2):
                    t = ip.tile([128, BUFLEN], f32r, name=f"ibuf_{s}_{kh}")
                    nc.vector.tensor_copy(t[:], zt[:].to_broadcast((128, BUFLEN)))
                    bufs[s, kh] = t

            taps = [(ky, kx, kh) for ky in range(3) for kx in range(3)
                    for kh in range(2)]

            def body():
                for img in range(BPC):
                    for s in range(NSTRIP):
                        ys = s * RS
                        y0 = max(ys - 1, 0)
                        y1 = min(ys + RS + 1, H)
                        nrows = y1 - y0
                        r0 = y0 - (ys - 1)  # buffer row of first loaded image row
                        for kh in range(2):
                            dst = (
                                bufs[s, kh][:, G + r0 * S: G + (r0 + nrows) * S]
                                .rearrange("p (r c) -> p r c", c=S)[:, :, 1:1 + Wd]
                            )
                            nc.sync.dma_start(
                                dst,
                                inp_d[img, kh * 128:(kh + 1) * 128, y0:y1, :]
                                .bitcast(f32r),
                            )
                        for mh in range(2):
                            psums = [
                                pp.tile([128, NPOS], f32, name="ps", tag="ps")
                                for _ in range(NT)
                            ]
                            for ti, (ky, kx, kh) in enumerate(taps):
                                w_ = wt[ky, kx, kh, mh]
                                dy, dx = ky - 1, kx - 1
                                for nt in range(NT):
                                    off = G + (1 + 4 * nt + dy) * S + dx
                                    nc.tensor.matmul(
                                        psums[nt][:], w_[:],
                                        bufs[s, kh][:, off:off + NPOS],
                                        start=(ti == 0), stop=(ti == len(taps) - 1),
                                    )
                            for nt in range(NT):
                                ot = op.tile([128, NPOS], f32, name="ot", tag="ot")
                                nc.vector.tensor_scalar_add(
                                    ot[:], psums[nt][:], biases[mh][:])
                                src = (ot[:].rearrange("p (r c) -> p r c", c=S)
                                       [:, :, 1:1 + Wd])
                                nc.sync.dma_start(
                                    out_d[img, mh * 128:(mh + 1) * 128,
                                          ys + 4 * nt: ys + 4 * nt + 4, :],
                                    src,
                                )

            if repeat > 1:
                with tc.For_i(0, repeat, 1):
                    body()
            else:
                body()

    nc.compile()
    return nc


_NC = None


def kernel(inp, W, bias):
    global _NC
    if _NC is None:
        _NC = build()
    inp = np.ascontiguousarray(np.asarray(inp, dtype=np.float32))
    w_t = np.ascontiguousarray(
        np.transpose(np.asarray(W, dtype=np.float32), (2, 3, 1, 0)))
    bias_r = np.ascontiguousarray(
        np.asarray(bias, dtype=np.float32).reshape(C, 1))
    in_maps = [
        {"inp": inp[c * BPC:(c + 1) * BPC], "w": w_t, "bias": bias_r}
        for c in range(NCORES)
    ]
    res = run_bass_kernel_spmd(_NC, in_maps, list(range(NCORES)))
    return np.concatenate(
        [res.results[c]["out"] for c in range(NCORES)], axis=0)


# revision 10
# speedup vs baseline: 1.9423x; 1.9423x over previous
"""Trainium2 Bass kernel for 3x3 same-padding Conv2d (B=16, C=256, H=W=112) + bias.

Strategy: data-parallel over batch (2 images per core on 8 NeuronCores).
Per core, implicit GEMM: activations live in SBUF in a zero-padded layout with
row stride 113 (one shared zero column between rows plus guard rows), so each
of the 9 conv taps is a matmul whose moving operand is the same buffer sliced
at a shifted offset. Contraction over 256 input channels = 2 partition tiles;
256 output channels = 2 PSUM tiles. Matmuls run in float32r (TF32-like,
1 cycle/row at N>=256, ~1.5e-4 rel err) fed directly by DMA.

The image is processed in 4 horizontal strips of 28 output rows (30 input
rows with halo). Each strip yields 7 PSUM accumulation groups of 452
positions (4 output rows x 113) per output-channel half; each group
accumulates 18 matmuls (9 taps x 2 input-channel halves). Bias is fused into
the PSUM->SBUF drain via tensor_scalar_add.
"""
import numpy as np

from concourse import bacc, bass, mybir, tile
from concourse.bass_utils import run_bass_kernel_spmd

B, C, H, Wd = 16, 256, 112, 112
NCORES = 8
BPC = B // NCORES        # images per core
S = 113                  # padded row stride (112 data + 1 shared zero col)
G = 4                    # guard zeros at buffer start
RS = 28                  # output rows per strip
NSTRIP = H // RS         # 4
IN_ROWS = RS + 2         # input rows per strip incl. halo
BUFLEN = G + IN_ROWS * S + 2   # 3396
NPOS = 4 * S             # 452 positions per matmul / psum group
NT = RS // 4             # 7 psum groups per strip
f32 = mybir.dt.float32
f32r = mybir.dt.float32r


def build(repeat: int = 1, no_in: bool = False, no_out: bool = False,
          no_mm: bool = False, in_engine: str = "scalar", out_engine: str = "sync",
          in_mode: str = "direct", out_contig: bool = False):
    nc = bacc.Bacc("TRN2", debug=False)
    inp_d = nc.dram_tensor("inp", [BPC, C, H, Wd], f32, kind="ExternalInput").ap()
    w_d = nc.dram_tensor("w", [3, 3, C, C], f32, kind="ExternalInput").ap()
    bias_d = nc.dram_tensor("bias", [C, 1], f32, kind="ExternalInput").ap()
    out_d = nc.dram_tensor("out", [BPC, C, H, Wd], f32, kind="ExternalOutput").ap()

    with tile.TileContext(nc) as tc:
        with (
            tc.tile_pool(name="wpool", bufs=1) as wp,
            tc.tile_pool(name="ipool", bufs=1) as ip,
            tc.tile_pool(name="opool", bufs=8) as op,
            tc.tile_pool(name="pspool", bufs=8, space=bass.MemorySpace.PSUM) as pp,
        ):
            # Weight tiles: lhsT[K=in_ch, M=out_ch] per (tap, in-half, out-half).
            wt = {}
            for ky in range(3):
                for kx in range(3):
                    for kh in range(2):
                        for mh in range(2):
                            t = wp.tile([128, 128], f32r, name=f"w_{ky}{kx}{kh}{mh}")
                            nc.sync.dma_start(
                                t[:],
                                w_d[ky, kx, kh * 128:(kh + 1) * 128,
                                    mh * 128:(mh + 1) * 128].bitcast(f32r),
                            )
                            wt[ky, kx, kh, mh] = t
            biases = []
            for mh in range(2):
                bt = wp.tile([128, 1], f32, name=f"bias_{mh}")
                nc.sync.dma_start(bt[:], bias_d[mh * 128:(mh + 1) * 128, :])
                biases.append(bt)

            # Input buffers: fixed per (strip, in-half) so the zero borders
            # (guards, shared zero columns, halo rows of first/last strip)
            # survive reuse across images; DMA only ever writes the interior.
            # Zeroing goes through a broadcast f32->f32r copy: walrus rejects
            # Memset on float32r, and every producer feeding an f32r matmul
            # must itself emit f32r ("rounded") output.
            zt = wp.tile([128, 1], f32, name="zero_src")
            nc.vector.memset(zt[:], 0.0)
            bufs = {}
            for s in range(NSTRIP):
                for kh in range(2):
                    t = ip.tile([128, BUFLEN], f32r, name=f"ibuf_{s}_{kh}")
                    nc.vector.tensor_copy(t[:], zt[:].to_broadcast((128, BUFLEN)))
                    bufs[s, kh] = t

            taps = [(ky, kx, kh) for ky in range(3) for kx in range(3)
                    for kh in range(2)]

            def body():
                for img in range(BPC):
                    for s in range(NSTRIP):
                        ys = s * RS
                        y0 = max(ys - 1, 0)
                        y1 = min(ys + RS + 1, H)
                        nrows = y1 - y0
                        r0 = y0 - (ys - 1)  # buffer row of first loaded image row
                        for kh in range(2):
                            if no_in:
                                continue
                            dst = (
                                bufs[s, kh][:, G + r0 * S: G + (r0 + nrows) * S]
                                .rearrange("p (r c) -> p r c", c=S)[:, :, 1:1 + Wd]
                            )
                            src = inp_d[img, kh * 128:(kh + 1) * 128, y0:y1, :]
                            if in_mode == "direct":
                                getattr(nc, in_engine).dma_start(
                                    dst, src.bitcast(f32r))
                            else:
                                # contiguous DMA into scratch at line rate, then
                                # DVE scatter into the padded (stride-113) layout
                                sc = op.tile([128, IN_ROWS * Wd], f32,
                                             name="iscr", tag="iscr", bufs=3)
                                getattr(nc, in_engine).dma_start(
                                    sc[:, :nrows * Wd],
                                    src.rearrange("p r c -> p (r c)"))
                                nc.vector.tensor_copy(
                                    dst,
                                    sc[:, :nrows * Wd]
                                    .rearrange("p (r c) -> p r c", c=Wd))
                        for mh in range(2):
                            psums = [
                                pp.tile([128, NPOS], f32, name="ps", tag="ps")
                                for _ in range(NT)
                            ] if not no_mm else None
                            if not no_mm:
                                for ti, (ky, kx, kh) in enumerate(taps):
                                    w_ = wt[ky, kx, kh, mh]
                                    dy, dx = ky - 1, kx - 1
                                    for nt in range(NT):
                                        off = G + (1 + 4 * nt + dy) * S + dx
                                        nc.tensor.matmul(
                                            psums[nt][:], w_[:],
                                            bufs[s, kh][:, off:off + NPOS],
                                            start=(ti == 0),
                                            stop=(ti == len(taps) - 1),
                                        )
                            for nt in range(NT):
                                if out_contig:
                                    # drop the border column during the PSUM
                                    # drain so both sides of the out-DMA are
                                    # fully contiguous
                                    ot = op.tile([128, 4 * Wd], f32,
                                                 name="ot", tag="ot")
                                    if no_mm:
                                        nc.vector.tensor_copy(
                                            ot[:], zt[:].to_broadcast((128, 4 * Wd)))
                                    else:
                                        nc.vector.tensor_scalar_add(
                                            ot[:].rearrange(
                                                "p (r c) -> p r c", c=Wd),
                                            psums[nt][:]
                                            .rearrange("p (r c) -> p r c", c=S)
                                            [:, :, 1:1 + Wd],
                                            biases[mh][:])
                                    dma_src = ot[:]
                                else:
                                    ot = op.tile([128, NPOS], f32,
                                                 name="ot", tag="ot")
                                    if no_mm:
                                        nc.vector.tensor_copy(
                                            ot[:], zt[:].to_broadcast((128, NPOS)))
                                    else:
                                        nc.vector.tensor_scalar_add(
                                            ot[:], psums[nt][:], biases[mh][:])
                                    dma_src = (ot[:]
                                               .rearrange("p (r c) -> p r c", c=S)
                                               [:, :, 1:1 + Wd])
                                if no_out:
                                    continue
                                getattr(nc, out_engine).dma_start(
                                    out_d[img, mh * 128:(mh + 1) * 128,
                                          ys + 4 * nt: ys + 4 * nt + 4, :]
                                    .rearrange("p r c -> p (r c)")
                                    if out_contig else
                                    out_d[img, mh * 128:(mh + 1) * 128,
                                          ys + 4 * nt: ys + 4 * nt + 4, :],
                                    dma_src,
                                )

            if repeat > 1:
                with tc.For_i(0, repeat, 1):
                    body()
            else:
                body()

    nc.compile()
    return nc


_NC = None


def kernel(inp, W, bias):
    global _NC
    if _NC is None:
        _NC = build()
    inp = np.ascontiguousarray(np.asarray(inp, dtype=np.float32))
    w_t = np.ascontiguousarray(
        np.transpose(np.asarray(W, dtype=np.float32), (2, 3, 1, 0)))
    bias_r = np.ascontiguousarray(
        np.asarray(bias, dtype=np.float32).reshape(C, 1))
    in_maps = [
        {"inp": inp[c * BPC:(c + 1) * BPC], "w": w_t, "bias": bias_r}
        for c in range(NCORES)
    ]
    res = run_bass_kernel_spmd(_NC, in_maps, list(range(NCORES)))
    return np.concatenate(
        [res.results[c]["out"] for c in range(NCORES)], axis=0)


# revision 11
# speedup vs baseline: 2.7036x; 1.3920x over previous
"""Trainium2 Bass kernel for 3x3 same-padding Conv2d (B=16, C=256, H=W=112) + bias.

Strategy: data-parallel over batch (2 images per core on 8 NeuronCores).
Per core, implicit GEMM: activations live in SBUF in a zero-padded layout with
row stride 113 (one shared zero column between rows plus guard rows), so each
of the 9 conv taps is a matmul whose moving operand is the same buffer sliced
at a shifted offset. Contraction over 256 input channels = 2 partition tiles;
256 output channels = 2 PSUM tiles. Matmuls run in float32r (TF32-like,
1 cycle/row at N>=256, ~1.5e-4 rel err) fed directly by DMA.

The image is processed in 4 horizontal strips of 28 output rows (30 input
rows with halo). Each strip yields 7 PSUM accumulation groups of 452
positions (4 output rows x 113) per output-channel half; each group
accumulates 18 matmuls (9 taps x 2 input-channel halves). Bias is fused into
the PSUM->SBUF drain via tensor_scalar_add.

Input DMAs issue on the scalar-engine HWDGE ring and output DMAs on the
sync-engine ring: sharing one ring serializes the sem-gated output stores
against input prefetch (head-of-line blocking) and measured 2.4x slower.
"""
import numpy as np

from concourse import bacc, bass, mybir, tile
from concourse.bass_utils import run_bass_kernel_spmd

B, C, H, Wd = 16, 256, 112, 112
NCORES = 8
BPC = B // NCORES        # images per core
S = 113                  # padded row stride (112 data + 1 shared zero col)
G = 4                    # guard zeros at buffer start
RS = 28                  # output rows per strip
NSTRIP = H // RS         # 4
IN_ROWS = RS + 2         # input rows per strip incl. halo
BUFLEN = G + IN_ROWS * S + 2   # 3396
NPOS = 4 * S             # 452 positions per matmul / psum group
NT = RS // 4             # 7 psum groups per strip
f32 = mybir.dt.float32
f32r = mybir.dt.float32r


def build(repeat: int = 1, no_in: bool = False, no_out: bool = False,
          no_mm: bool = False, in_engine: str = "scalar", out_engine: str = "sync",
          in_mode: str = "direct", out_contig: bool = False):
    nc = bacc.Bacc("TRN2", debug=False)
    inp_d = nc.dram_tensor("inp", [BPC, C, H, Wd], f32, kind="ExternalInput").ap()
    w_d = nc.dram_tensor("w", [3, 3, C, C], f32, kind="ExternalInput").ap()
    bias_d = nc.dram_tensor("bias", [C, 1], f32, kind="ExternalInput").ap()
    out_d = nc.dram_tensor("out", [BPC, C, H, Wd], f32, kind="ExternalOutput").ap()

    with tile.TileContext(nc) as tc:
        with (
            tc.tile_pool(name="wpool", bufs=1) as wp,
            tc.tile_pool(name="ipool", bufs=1) as ip,
            tc.tile_pool(name="opool", bufs=8) as op,
            tc.tile_pool(name="pspool", bufs=8, space=bass.MemorySpace.PSUM) as pp,
        ):
            # Weight tiles: lhsT[K=in_ch, M=out_ch] per (tap, in-half, out-half).
            wt = {}
            for ky in range(3):
                for kx in range(3):
                    for kh in range(2):
                        for mh in range(2):
                            t = wp.tile([128, 128], f32r, name=f"w_{ky}{kx}{kh}{mh}")
                            nc.sync.dma_start(
                                t[:],
                                w_d[ky, kx, kh * 128:(kh + 1) * 128,
                                    mh * 128:(mh + 1) * 128].bitcast(f32r),
                            )
                            wt[ky, kx, kh, mh] = t
            biases = []
            for mh in range(2):
                bt = wp.tile([128, 1], f32, name=f"bias_{mh}")
                nc.sync.dma_start(bt[:], bias_d[mh * 128:(mh + 1) * 128, :])
                biases.append(bt)

            # Input buffers: fixed per (strip, in-half) so the zero borders
            # (guards, shared zero columns, halo rows of first/last strip)
            # survive reuse across images; DMA only ever writes the interior.
            # Zeroing goes through a broadcast f32->f32r copy: walrus rejects
            # Memset on float32r, and every producer feeding an f32r matmul
            # must itself emit f32r ("rounded") output.
            zt = wp.tile([128, 1], f32, name="zero_src")
            nc.vector.memset(zt[:], 0.0)
            bufs = {}
            for s in range(NSTRIP):
                for kh in range(2):
                    t = ip.tile([128, BUFLEN], f32r, name=f"ibuf_{s}_{kh}")
                    nc.vector.tensor_copy(t[:], zt[:].to_broadcast((128, BUFLEN)))
                    bufs[s, kh] = t

            taps = [(ky, kx, kh) for ky in range(3) for kx in range(3)
                    for kh in range(2)]

            def body():
                for img in range(BPC):
                    for s in range(NSTRIP):
                        ys = s * RS
                        y0 = max(ys - 1, 0)
                        y1 = min(ys + RS + 1, H)
                        nrows = y1 - y0
                        r0 = y0 - (ys - 1)  # buffer row of first loaded image row
                        for kh in range(2):
                            if no_in:
                                continue
                            dst = (
                                bufs[s, kh][:, G + r0 * S: G + (r0 + nrows) * S]
                                .rearrange("p (r c) -> p r c", c=S)[:, :, 1:1 + Wd]
                            )
                            src = inp_d[img, kh * 128:(kh + 1) * 128, y0:y1, :]
                            if in_mode == "direct":
                                getattr(nc, in_engine).dma_start(
                                    dst, src.bitcast(f32r))
                            else:
                                # contiguous DMA into scratch at line rate, then
                                # DVE scatter into the padded (stride-113) layout
                                sc = op.tile([128, IN_ROWS * Wd], f32,
                                             name="iscr", tag="iscr", bufs=3)
                                getattr(nc, in_engine).dma_start(
                                    sc[:, :nrows * Wd],
                                    src.rearrange("p r c -> p (r c)"))
                                nc.vector.tensor_copy(
                                    dst,
                                    sc[:, :nrows * Wd]
                                    .rearrange("p (r c) -> p r c", c=Wd))
                        for mh in range(2):
                            psums = [
                                pp.tile([128, NPOS], f32, name="ps", tag="ps")
                                for _ in range(NT)
                            ] if not no_mm else None
                            if not no_mm:
                                for ti, (ky, kx, kh) in enumerate(taps):
                                    w_ = wt[ky, kx, kh, mh]
                                    dy, dx = ky - 1, kx - 1
                                    for nt in range(NT):
                                        off = G + (1 + 4 * nt + dy) * S + dx
                                        nc.tensor.matmul(
                                            psums[nt][:], w_[:],
                                            bufs[s, kh][:, off:off + NPOS],
                                            start=(ti == 0),
                                            stop=(ti == len(taps) - 1),
                                        )
                            for nt in range(NT):
                                if out_contig:
                                    # drop the border column during the PSUM
                                    # drain so both sides of the out-DMA are
                                    # fully contiguous
                                    ot = op.tile([128, 4 * Wd], f32,
                                                 name="ot", tag="ot")
                                    if no_mm:
                                        nc.vector.tensor_copy(
                                            ot[:], zt[:].to_broadcast((128, 4 * Wd)))
                                    else:
                                        nc.vector.tensor_scalar_add(
                                            ot[:].rearrange(
                                                "p (r c) -> p r c", c=Wd),
                                            psums[nt][:]
                                            .rearrange("p (r c) -> p r c", c=S)
                                            [:, :, 1:1 + Wd],
                                            biases[mh][:])
                                    dma_src = ot[:]
                                else:
                                    ot = op.tile([128, NPOS], f32,
                                                 name="ot", tag="ot")
                                    if no_mm:
                                        nc.vector.tensor_copy(
                                            ot[:], zt[:].to_broadcast((128, NPOS)))
                                    else:
                                        nc.vector.tensor_scalar_add(
                                            ot[:], psums[nt][:], biases[mh][:])
                                    dma_src = (ot[:]
                                               .rearrange("p (r c) -> p r c", c=S)
                                               [:, :, 1:1 + Wd])
                                if no_out:
                                    continue
                                getattr(nc, out_engine).dma_start(
                                    out_d[img, mh * 128:(mh + 1) * 128,
                                          ys + 4 * nt: ys + 4 * nt + 4, :]
                                    .rearrange("p r c -> p (r c)")
                                    if out_contig else
                                    out_d[img, mh * 128:(mh + 1) * 128,
                                          ys + 4 * nt: ys + 4 * nt + 4, :],
                                    dma_src,
                                )

            if repeat > 1:
                with tc.For_i(0, repeat, 1):
                    body()
            else:
                body()

    nc.compile()
    return nc


_NC = None


def kernel(inp, W, bias):
    global _NC
    if _NC is None:
        _NC = build()
    inp = np.ascontiguousarray(np.asarray(inp, dtype=np.float32))
    w_t = np.ascontiguousarray(
        np.transpose(np.asarray(W, dtype=np.float32), (2, 3, 1, 0)))
    bias_r = np.ascontiguousarray(
        np.asarray(bias, dtype=np.float32).reshape(C, 1))
    in_maps = [
        {"inp": inp[c * BPC:(c + 1) * BPC], "w": w_t, "bias": bias_r}
        for c in range(NCORES)
    ]
    res = run_bass_kernel_spmd(_NC, in_maps, list(range(NCORES)))
    return np.concatenate(
        [res.results[c]["out"] for c in range(NCORES)], axis=0)
